# revision 37
# baseline (speedup 1.0000x reference)
"""ChemProp message-to-node + MLP kernel for 8 TRN2 NeuronCores.

Strategy (no collectives needed):
  - Host assigns nodes to cores by global degree rank, round-robin, so
    each core receives exactly the edges destined for its nodes and all
    cores see near-identical degree sequences (minimal padding). Edge
    features are pre-permuted into a "degree-slot" layout so the
    device-side segment-sum is pure contiguous streaming adds.
  - Node groups of <=512 (one PSUM window). Within a group, nodes are
    sorted by degree (desc). Slot d holds the d-th edge of every node
    with degree > d, so each slot is a contiguous run of columns that
    adds elementwise into a prefix of the group's message accumulator.
  - Layout is feature-major ([256, cols] split into 2x128 partitions) so
    the MLP runs without any transposes: hidden^T = W1^T @ cat^T etc.
  - Segment-sum: feature-ptile 0 accumulates in PSUM via identity
    matmuls (TensorE, exactly one start=True per window), ptile 1
    accumulates on DVE in an SBUF f32 tile; the last TAIL_PE_GROUPS
    groups run both ptiles on PE to shorten the pipeline tail. The MLP
    runs in bf16 with f32 PSUM accumulation; stream/out DMAs are issued
    from separate engine queues to avoid head-of-line blocking.
  - Per-core output slice is returned feature-major (bf16); host
    transposes, un-permutes, casts to f32 and concatenates.
"""

import numpy as np
import ml_dtypes

import concourse.bacc as bacc
import concourse.mybir as mybir
import concourse.tile as tile
from concourse.bass_utils import run_bass_kernel_spmd
from concourse.masks import make_identity

NC = 8          # cores
GRP = 512       # nodes per group (one PSUM window)
CHUNK = 2048    # stream-chunk columns
STREAM_BUFS = 10
MSG_BUFS = 2
ACC_BUFS = 2
PSUM_MSG_BUFS = 2
HID_BUFS = 2
SKIP_MLP = False     # diagnostic: drop MLP+out stages (timing only)
DIAG_MSG = False     # diagnostic: output msgb directly (needs OUT_BF16)
OUT_BF16 = True      # device writes bf16 output; host casts back to f32
STREAM_BF16 = True   # v2: bf16 edge stream + TensorE adds; False: f32 + DVE
DVE_PTILE = True     # ptile1 segment-sum on DVE (False: all on PE)
GPSIMD_DMA = True    # issue rT/weight DMAs from gpsimd (False: sync)
TAIL_PE_GROUPS = 4   # last N groups: both ptiles on PE (kills tail latency)
TAIL_CAPS = ()       # split the last full group into these sizes (sum=GRP)
TAIL_PATTERN = ()    # explicit pe/dve pattern for the last len() groups
RT_BATCH = 1         # groups per rT load strip
OUT_BATCH = 1        # groups per out store strip

BF16 = mybir.dt.bfloat16
F32 = mybir.dt.float32
NP_BF16 = ml_dtypes.bfloat16


# ----------------------------------------------------------------- host side
def _preprocess(r, h, nbrs):
    """Build per-core streams/permutations. Returns layout + per-core arrays."""
    n_nodes, Fdim = r.shape
    n_edges = h.shape[0]
    npc = n_nodes // NC
    caps = [GRP] * (npc // GRP)
    rem = npc % GRP
    if TAIL_CAPS:
        assert sum(TAIL_CAPS) == GRP
        caps = caps[:-1] + list(TAIL_CAPS)
    if rem:
        caps.append(rem)
    grp_lo = np.concatenate([[0], np.cumsum(caps)]).astype(np.int64)
    ngrp = len(caps)

    dst = nbrs[:, 0].astype(np.int64)
    deg_flat = np.bincount(dst, minlength=n_nodes)
    order = np.argsort(dst, kind="stable")          # edges sorted by dest
    starts = np.zeros(n_nodes + 1, dtype=np.int64)
    np.cumsum(deg_flat, out=starts[1:])

    # Node -> (core, position) assignment: global degree rank, round-robin
    # over cores (so all cores see near-identical degree sequences -> minimal
    # cross-core slot padding), then round-robin over groups within the core
    # (so each group has a heterogeneous degree mix); within a group,
    # positions are filled in degree-desc order (the slot-prefix property).
    rank = np.argsort(-deg_flat, kind="stable")     # rank idx -> global node
    node_ids = np.zeros((NC, npc), dtype=np.int64)  # position -> global node
    deg_sorted = np.zeros((NC, npc), dtype=np.int64)
    for c in range(NC):
        ids_q = rank[c::NC]                          # degree-desc for core c
        fill = [0] * ngrp
        for q in range(npc):
            g = q % ngrp
            while fill[g] == caps[g]:
                g = (g + 1) % ngrp
            pos = int(grp_lo[g]) + fill[g]
            fill[g] += 1
            node_ids[c, pos] = ids_q[q]
        deg_sorted[c] = deg_flat[node_ids[c]]

    # regularized slot widths K[g][d] = max over cores of #nodes with deg > d
    # (slot 0 forced to full group width so every msg column is initialized)
    K = []
    slot_off = []
    off = 0
    for g in range(ngrp):
        lo = int(grp_lo[g])
        hi = int(grp_lo[g + 1])
        w = hi - lo
        degs = deg_sorted[:, lo:hi]                  # [NC, w]
        dmax = max(int(degs.max()), 1)
        counts = (degs[:, :, None] > np.arange(dmax)[None, None, :]).sum(1)
        Kg = counts.max(0)                           # [dmax]
        Kg[0] = w
        offs = off + np.concatenate([[0], np.cumsum(Kg)])
        K.append(Kg.astype(np.int64))
        slot_off.append(offs.astype(np.int64))
        off = int(offs[-1])
    cols = off

    # col -> edge id (n_edges = zero pad), per core
    col_edge = np.full((NC, cols), n_edges, dtype=np.int64)
    for c in range(NC):
        for g in range(ngrp):
            lo = int(grp_lo[g])
            degs_g = deg_sorted[c, lo:int(grp_lo[g + 1])]
            for d in range(len(K[g])):
                kcd = int((degs_g > d).sum())
                if kcd == 0:
                    continue
                nodes = node_ids[c, lo:lo + kcd]
                c0 = slot_off[g][d]
                col_edge[c, c0:c0 + kcd] = order[starts[nodes] + d]

    return {
        "npc": npc, "ngrp": ngrp, "cols": cols, "F": Fdim,
        "K": K, "slot_off": slot_off, "node_ids": node_ids,
        "col_edge": col_edge, "grp_lo": grp_lo,
    }


def _build_streams(h, r, lay):
    """Materialize per-core device input arrays."""
    n_edges, Fdim = h.shape
    npc, cols = lay["npc"], lay["cols"]
    fp = Fdim // 128                                 # feature partition-tiles
    sdt = NP_BF16 if STREAM_BF16 else np.float32

    h_aug = np.zeros((n_edges + 1, Fdim), dtype=sdt)
    h_aug[:n_edges] = h.astype(sdt)
    hs, rT = [], []
    for c in range(NC):
        block = h_aug[lay["col_edge"][c]]            # [cols, F]
        hs.append(np.ascontiguousarray(block.T).reshape(fp, 128, cols))
        rc = r[lay["node_ids"][c]].astype(NP_BF16)
        rT.append(np.ascontiguousarray(rc.T).reshape(fp, 128, npc))
    return hs, rT


# --------------------------------------------------------------- device side
def _pieces_for_group(lay, g):
    """Yield (src_col0, dst_col0, length) spans for group g's slot adds."""
    for d in range(len(lay["K"][g])):
        c0 = int(lay["slot_off"][g][d])
        k = int(lay["K"][g][d])
        yield c0, 0, k


def _build_graph(lay, Fdim, H, Fout):
    npc, ngrp, cols = lay["npc"], lay["ngrp"], lay["cols"]
    fp = Fdim // 128          # 2 feature ptiles
    kt_n = (2 * Fdim) // 128  # 4 k-chunks for W1
    ht_n = H // 128           # 4 hidden ptiles
    ot_n = Fout // 128        # 2 output ptiles
    sdt = BF16 if STREAM_BF16 else F32

    nc = bacc.Bacc(None, target_bir_lowering=False)
    hs_p = nc.declare_dram_parameter("hs", [fp, 128, cols], sdt, isOutput=False)
    rT_p = nc.declare_dram_parameter("rT", [fp, 128, npc], BF16, isOutput=False)
    w1_p = nc.declare_dram_parameter("W1", [kt_n, 128, H], BF16, isOutput=False)
    w2_p = nc.declare_dram_parameter("W2", [ht_n, 128, Fout], BF16, isOutput=False)
    out_dt = BF16 if OUT_BF16 else F32
    out_p = nc.declare_dram_parameter("out", [ot_n, 128, npc], out_dt,
                                      isOutput=True)

    n_chunks = (cols + CHUNK - 1) // CHUNK

    with tile.TileContext(nc) as tc:
        with (
            tc.tile_pool(name="const", bufs=1) as const_pool,
            tc.tile_pool(name="stream", bufs=STREAM_BUFS) as stream_pool,
            tc.tile_pool(name="msgp", bufs=PSUM_MSG_BUFS, space="PSUM") as msg_psum_pool,
            tc.tile_pool(name="msgb", bufs=MSG_BUFS) as msg_pool,
            tc.tile_pool(name="acc", bufs=ACC_BUFS) as acc_pool,
            tc.tile_pool(name="rb", bufs=2) as r_pool,
            tc.tile_pool(name="mlp1p", bufs=2, space="PSUM") as mlp1_psum_pool,
            tc.tile_pool(name="mlp2p", bufs=2, space="PSUM") as mlp2_psum_pool,
            tc.tile_pool(name="hid", bufs=HID_BUFS) as hid_pool,
            tc.tile_pool(name="osb", bufs=2) as out_pool,
        ):
            # weights resident in SBUF
            w1_sb = []
            for k in range(kt_n):
                t = const_pool.tile([128, H], BF16, tag=f"w1_{k}")
                (nc.gpsimd if GPSIMD_DMA else nc.sync).dma_start(out=t[:], in_=w1_p[k])
                w1_sb.append(t)
            w2_sb = []
            for k in range(ht_n):
                t = const_pool.tile([128, Fout], BF16, tag=f"w2_{k}")
                (nc.gpsimd if GPSIMD_DMA else nc.sync).dma_start(out=t[:], in_=w2_p[k])
                w2_sb.append(t)
            ident = None
            if STREAM_BF16:
                ident = const_pool.tile([128, 128], BF16, tag="ident")
                make_identity(nc, ident)

            chunk_tiles = [[None] * n_chunks for _ in range(fp)]

            def get_chunk(p, ci):
                if chunk_tiles[p][ci] is None:
                    w = min(CHUNK, cols - ci * CHUNK)
                    t = stream_pool.tile([128, w], sdt, tag=f"hs{p}")
                    nc.sync.dma_start(
                        out=t[:], in_=hs_p[p, :, ci * CHUNK:ci * CHUNK + w])
                    chunk_tiles[p][ci] = t
                return chunk_tiles[p][ci]

            for g in range(ngrp):
                lo = int(lay["grp_lo"][g])
                w_g = int(lay["grp_lo"][g + 1]) - lo

                # ---- segment-sum for this group's nodes
                pieces = []   # (slot, chunk, src_off, dst_off, len)
                for d, (c0, d0, k) in enumerate(_pieces_for_group(lay, g)):
                    # split on chunk boundaries
                    s = c0
                    while s < c0 + k:
                        ci = s // CHUNK
                        e = min(c0 + k, (ci + 1) * CHUNK)
                        pieces.append((d, ci, s - ci * CHUNK, d0 + (s - c0),
                                       e - s))
                        s = e

                msgb = []
                for p in range(fp):
                    if STREAM_BF16:
                        mb = msg_pool.tile([128, w_g], BF16, tag=f"mb{p}")
                        tail_i = g - (ngrp - len(TAIL_PATTERN)) \
                            if TAIL_PATTERN else -1
                        on_pe = (not DVE_PTILE) or p % 2 == 0
                        if tail_i >= 0:
                            on_pe = on_pe or TAIL_PATTERN[tail_i] == "pe"
                        elif g >= ngrp - TAIL_PE_GROUPS:
                            on_pe = True
                        if on_pe:
                            # PE path: identity matmuls accumulate in PSUM
                            ps = msg_psum_pool.tile([128, w_g], F32,
                                                    space="PSUM", tag=f"mp{p}")
                            for i, (d, ci, o0, dj, ln) in enumerate(pieces):
                                src = get_chunk(p, ci)
                                # exactly ONE start=True per PSUM window: a
                                # second one resets the bank's has_written
                                # bits and drops prior fragments' data.
                                # Untouched columns first-touch via
                                # has_written=0 on their first start=False.
                                nc.tensor.matmul(
                                    out=ps[:, dj:dj + ln],
                                    lhsT=ident[:],
                                    rhs=src[:, o0:o0 + ln],
                                    start=(i == 0),
                                    stop=(i == len(pieces) - 1),
                                    skip_group_check=True,
                                )
                            nc.scalar.activation(
                                mb[:], ps[:], mybir.ActivationFunctionType.Copy)
                        else:
                            # DVE path: slot-0 copy initializes (full width),
                            # later slots accumulate in an SBUF f32 tile
                            acc = acc_pool.tile([128, w_g], F32, tag=f"ac{p}")
                            for (d, ci, o0, dj, ln) in pieces:
                                src = get_chunk(p, ci)
                                if d == 0:
                                    nc.vector.tensor_copy(
                                        out=acc[:, dj:dj + ln],
                                        in_=src[:, o0:o0 + ln])
                                else:
                                    nc.vector.tensor_tensor(
                                        out=acc[:, dj:dj + ln],
                                        in0=acc[:, dj:dj + ln],
                                        in1=src[:, o0:o0 + ln],
                                        op=mybir.AluOpType.add)
                            nc.vector.tensor_copy(out=mb[:], in_=acc[:])
                        msgb.append(mb)
                    else:
                        acc = msg_pool.tile([128, w_g], F32, tag=f"macc{p}")
                        nc.any.memset(acc[:], 0.0)
                        for (d, ci, o0, dj, ln) in pieces:
                            src = get_chunk(p, ci)
                            nc.vector.tensor_tensor(
                                out=acc[:, dj:dj + ln], in0=acc[:, dj:dj + ln],
                                in1=src[:, o0:o0 + ln], op=mybir.AluOpType.add)
                        mb = msg_pool.tile([128, w_g], BF16, tag=f"mb{p}")
                        nc.vector.tensor_copy(out=mb[:], in_=acc[:])
                        msgb.append(mb)

                if DIAG_MSG:
                    for ot in range(ot_n):
                        nc.sync.dma_start(out=out_p[ot, :, lo:lo + w_g],
                                          in_=msgb[ot][:])
                    continue
                if SKIP_MLP:
                    continue
                # ---- r slice (bf16, already permuted on host); loaded in
                # RT_BATCH-group strips so DMA descriptors stay >= 4KB
                if g % RT_BATCH == 0:
                    b_lo = lo
                    b_hi = int(lay["grp_lo"][min(g + RT_BATCH, ngrp)])
                    rb_strip = []
                    for p in range(fp):
                        t = r_pool.tile([128, b_hi - b_lo], BF16, tag=f"rb{p}")
                        (nc.gpsimd if GPSIMD_DMA else nc.sync).dma_start(
                            out=t[:], in_=rT_p[p, :, b_lo:b_hi])
                        rb_strip.append(t)
                    rb_base = b_lo
                rb = [t[:, lo - rb_base:lo - rb_base + w_g] for t in rb_strip]
                cat = rb + msgb  # k-chunk order matches W1 rows

                # ---- MLP: hidden^T = relu(W1^T @ cat^T)
                hid = []
                for ht in range(ht_n):
                    ps = mlp1_psum_pool.tile([128, w_g], F32, space="PSUM",
                                             tag="mlp1")
                    for k in range(kt_n):
                        nc.tensor.matmul(
                            out=ps[:],
                            lhsT=w1_sb[k][:, ht * 128:(ht + 1) * 128],
                            rhs=cat[k][:],
                            start=(k == 0), stop=(k == kt_n - 1))
                    hb = hid_pool.tile([128, w_g], BF16, tag=f"h{ht}")
                    nc.scalar.activation(
                        hb[:], ps[:], mybir.ActivationFunctionType.Relu)
                    hid.append(hb)

                # ---- out^T = W2^T @ hidden^T
                for ot in range(ot_n):
                    ps = mlp2_psum_pool.tile([128, w_g], F32, space="PSUM",
                                             tag="mlp2")
                    for k in range(ht_n):
                        nc.tensor.matmul(
                            out=ps[:],
                            lhsT=w2_sb[k][:, ot * 128:(ot + 1) * 128],
                            rhs=hid[k][:],
                            start=(k == 0), stop=(k == ht_n - 1))
                    if g % OUT_BATCH == 0 and ot == 0:
                        ob_lo = lo
                        ob_hi = int(lay["grp_lo"][min(g + OUT_BATCH, ngrp)])
                        ob_strips = []
                        for o in range(ot_n):
                            ob_t = out_pool.tile([128, ob_hi - ob_lo],
                                                 out_dt, tag=f"o{o}")
                            ob_strips.append(ob_t)
                    nc.scalar.activation(
                        ob_strips[ot][:, lo - ob_lo:lo - ob_lo + w_g],
                        ps[:], mybir.ActivationFunctionType.Copy)
                    if g % OUT_BATCH == OUT_BATCH - 1 or g == ngrp - 1:
                        nc.scalar.dma_start(
                            out=out_p[ot, :, ob_lo:ob_lo + ob_strips[ot].shape[1]],
                            in_=ob_strips[ot][:])

    nc.finalize()
    return nc


# ----------------------------------------------------------------- interface
def prepare(r, h, nbrs, W1, W2):
    """Preprocess inputs + build the Bass graph. Returns everything needed
    to run and to assemble the output."""
    r = np.asarray(r, dtype=np.float32)
    h = np.asarray(h, dtype=np.float32)
    nbrs = np.asarray(nbrs)
    W1 = np.asarray(W1, dtype=np.float32)
    W2 = np.asarray(W2, dtype=np.float32)

    n_nodes, Fdim = r.shape
    H = W1.shape[1]
    Fout = W2.shape[1]

    lay = _preprocess(r, h, nbrs)
    hs, rT = _build_streams(h, r, lay)
    w1d = np.ascontiguousarray(W1.astype(NP_BF16)).reshape(-1, 128, H)
    w2d = np.ascontiguousarray(W2.astype(NP_BF16)).reshape(-1, 128, Fout)

    nc = _build_graph(lay, Fdim, H, Fout)
    in_maps = [
        {"hs": hs[c], "rT": rT[c], "W1": w1d, "W2": w2d} for c in range(NC)
    ]
    return {"nc": nc, "in_maps": in_maps, "lay": lay,
            "n_nodes": n_nodes, "Fout": Fout}


def assemble(prep, results):
    lay = prep["lay"]
    n_nodes, Fout = prep["n_nodes"], prep["Fout"]
    npc = lay["npc"]
    out = np.zeros((n_nodes, Fout), dtype=np.float32)
    for c in range(NC):
        o = np.asarray(results[c]["out"]).reshape(Fout, npc)
        out[lay["node_ids"][c]] = o.T.astype(np.float32)
    return out


def kernel(r, h, nbrs, W1, W2):
    prep = prepare(r, h, nbrs, W1, W2)
    res = run_bass_kernel_spmd(prep["nc"], prep["in_maps"],
                               core_ids=list(range(NC)))
    return assemble(prep, res.results)
